# revision 11
# baseline (speedup 1.0000x reference)
"""DCNv2 (modulated deformable conv) Trainium2 kernel.

8 cores = 4 batch samples x 2 image halves. Per core:
  1. Offset conv on PE (fp32): om[27, 8192].
  2. Hat planes (DVE/ACT): A[d][tap,pix] = relu(1-|dy-(d-2)|)*sigmoid(m),
     t[tap,pix136] = dx + 3 + kx + j. Round-trip DRAM for affine replication.
  3. Per row: M[125, 1224] = relu(1 - |x_l - t|) * A  (partition = x_l*5+d).
  4. Stage-1 matmuls: host-prebuilt x-window stationaries [125, 64] x M
     -> v[64, (k, 17w+j)] = exact bilinear samples * mask (|offset|<=2).
  5. Stage-2: out[o,.] += W_k.T @ v_k over 9 taps; bias on evac.
"""
import sys
sys.path.insert(0, "/opt/trn_rl_repo")
import numpy as np
import concourse.bass as bass
import concourse.tile as tile
from concourse import bacc, mybir
from concourse.bass_utils import run_bass_kernel_spmd

F32, BF16 = mybir.dt.float32, mybir.dt.bfloat16
MUL, ADD = mybir.AluOpType.mult, mybir.AluOpType.add

B, C, O, H, W = 4, 64, 128, 128, 128
K = 9
HH = 64
P = HH * W
CW, NW, XW, ND = 17, 8, 25, 5
WJ = NW * CW           # 136
N1 = K * WJ            # 1224
RB = 4                 # rows per stage-2 block
NCORES = 8

_cache = {}


def _ap(base, dims):
    """Manual AP: keep base partition dim, replace free dims."""
    return bass.AP(base.tensor, base.offset, [base.ap[0]] + dims)


def build_bass(debug=False):
    nc = bacc.Bacc("TRN2", target_bir_lowering=False, debug=False,
                   num_devices=NCORES)
    dp = lambda n, s, dt, out=False: nc.dram_tensor(
        n, s, dt, kind="ExternalOutput" if out else "ExternalInput").ap()

    feat_d = dp("feat", [2 * C, 66 * 130], F32)
    xwin_d = dp("xwin", [128, 66 * NW * C], BF16)
    womr_d = dp("womr", [2 * C, 9 * 27], F32)
    w2r_d = dp("w2r", [C, K * O], BF16)
    bias_d = dp("bias", [O, 1], F32)
    bom_d = dp("bom", [27, 1], F32)
    kx3_d = dp("kx3", [36, 1], F32)
    xlb_d = dp("xlb", [125, 1], F32)
    dbias_d = dp("dbias", [36, 5], F32)
    zb125_d = dp("zb125", [125, 1], F32)
    out_d = dp("out", [O, P], F32, out=True)
    dbg = {}
    if debug:
        dbg["om"] = dp("dbg_om", [27, P], F32, out=True)
        dbg["M"] = dp("dbg_M", [125, N1], F32, out=True)
        dbg["v"] = dp("dbg_v", [C, N1], F32, out=True)

    omdram = nc.dram_tensor("omdram", [27, P], F32).ap()
    adram = nc.dram_tensor("adram", [ND * K * HH * WJ], BF16).ap()
    tdram = nc.dram_tensor("tdram", [K * HH * WJ], BF16).ap()
    a4 = adram.rearrange("(d k r c) -> d k r c", d=ND, k=K, r=HH)
    t3 = tdram.rearrange("(k r c) -> k r c", k=K, r=HH)

    with tile.TileContext(nc) as tc:
        with (
            tc.tile_pool(name="big", bufs=1) as big,
            tc.tile_pool(name="io", bufs=2) as io,
            tc.tile_pool(name="mpool", bufs=2) as mpool,
            tc.tile_pool(name="vpool", bufs=6) as vpool,
            tc.tile_pool(name="ps1", bufs=2, space="PSUM") as ps1,
            tc.tile_pool(name="ps2", bufs=4, space="PSUM") as ps2,
            tc.tile_pool(name="small", bufs=1) as small,
        ):
            womr = small.tile([2 * C, 9 * 27], F32)
            nc.sync.dma_start(out=womr[:], in_=womr_d[:])
            w2r = small.tile([C, K * O], BF16)
            nc.sync.dma_start(out=w2r[:], in_=w2r_d[:])
            bias = small.tile([O, 1], F32)
            nc.sync.dma_start(out=bias[:], in_=bias_d[:])
            bom = small.tile([27, 1], F32)
            nc.sync.dma_start(out=bom[:], in_=bom_d[:])
            kx3 = small.tile([36, 1], F32)
            nc.sync.dma_start(out=kx3[:], in_=kx3_d[:])
            xlb = small.tile([125, 1], F32)
            nc.sync.dma_start(out=xlb[:], in_=xlb_d[:])
            dbias = small.tile([36, 5], F32)
            nc.sync.dma_start(out=dbias[:], in_=dbias_d[:])
            zb125 = small.tile([125, 1], F32)
            nc.sync.dma_start(out=zb125[:], in_=zb125_d[:])

            feat = big.tile([2 * C, 66 * 130], F32)
            nc.sync.dma_start(out=feat[:], in_=feat_d[:])

            # ---- 1. offset conv ----
            for ch in range(16):
                r0 = ch * 4
                pom = ps1.tile([27, 512], F32, tag="pom")
                for t9 in range(9):
                    dy, dx = t9 // 3, t9 % 3
                    base = feat[:, (r0 + dy) * 130 + dx:(r0 + dy) * 130 + dx + 1]
                    rhs = _ap(base, [[130, 4], [1, 128]])
                    nc.tensor.matmul(pom[:], womr[:, t9 * 27:(t9 + 1) * 27],
                                     rhs, start=(t9 == 0), stop=(t9 == 8))
                omc = io.tile([27, 512], F32, tag="omc")
                nc.vector.tensor_scalar_add(omc[:], pom[:], bom[:])
                nc.sync.dma_start(out=omdram[:, ch * 512:(ch + 1) * 512],
                                  in_=omc[:])
            if debug:
                nc.sync.dma_start(out=dbg["om"][:], in_=omdram[:])

            # ---- 2. fat planes [36, 2056] ----
            dyf = big.tile([36, 2056], F32)
            dxf = big.tile([36, 2056], F32)
            msf = big.tile([36, 2056], F32)
            for f in (dyf, dxf, msf):
                nc.vector.memset(f[:], 0.0)
            for q in range(4):
                pix = slice(q * 2048, (q + 1) * 2048)
                nc.sync.dma_start(out=dyf[q * 9:(q + 1) * 9, 0:2048],
                                  in_=omdram[0:18:2, pix])
                nc.sync.dma_start(out=dxf[q * 9:(q + 1) * 9, 0:2048],
                                  in_=omdram[1:18:2, pix])
                nc.sync.dma_start(out=msf[q * 9:(q + 1) * 9, 0:2048],
                                  in_=omdram[18:27, pix])
            nc.scalar.activation(msf[:, 0:2048], msf[:, 0:2048],
                                 mybir.ActivationFunctionType.Sigmoid,
                                 bias=dbias[:, 2:3])
            nc.vector.memset(msf[:, 2048:2056], 0.0)

            # t136 = dx (136-layout) -> tdram (bf16); integer part added later
            dxv = _ap(dxf[:], [[128, 16], [17, 8], [1, 17]])
            t136b = big.tile([36, 2176], BF16)
            nc.vector.tensor_copy(t136b[:].rearrange(
                "p (a b c) -> p a b c", a=16, b=8, c=17), dxv)
            for q in range(4):
                nc.sync.dma_start(
                    out=t3[:, q * 16:(q + 1) * 16, :],
                    in_=t136b[q * 9:(q + 1) * 9, :].rearrange(
                        "p (r c) -> p r c", r=16))

            # A planes -> adram (bf16)
            msv = _ap(msf[:], [[128, 16], [17, 8], [1, 17]])
            for d5 in range(ND):
                ab = big.tile([36, 2176], F32, tag="aplane")
                dyv = _ap(dyf[:], [[128, 16], [17, 8], [1, 17]])
                ab4 = ab[:].rearrange("p (a b c) -> p a b c", a=16, b=8, c=17)
                nc.scalar.activation(ab4, dyv,
                                     mybir.ActivationFunctionType.Abs,
                                     bias=dbias[:, d5:d5 + 1], scale=1.0)
                nc.vector.tensor_scalar(ab[:], ab[:], -1.0, 1.0,
                                        op0=MUL, op1=ADD)
                nc.vector.tensor_scalar_max(ab[:], ab[:], 0.0)
                abb = big.tile([36, 2176], BF16, tag="aplaneb")
                abb4 = abb[:].rearrange("p (a b c) -> p a b c", a=16, b=8, c=17)
                nc.vector.tensor_tensor(abb4, ab4, msv, op=MUL)
                for q in range(4):
                    nc.sync.dma_start(
                        out=a4[d5, :, q * 16:(q + 1) * 16, :],
                        in_=abb[q * 9:(q + 1) * 9, :].rearrange(
                            "p (r c) -> p r c", r=16))

            jkx = big.tile([125, N1], BF16)
            nc.gpsimd.iota(jkx[:], pattern=[[0, 3], [1, 3], [0, 8], [1, 17]],
                           base=3, channel_multiplier=0,
                           allow_small_or_imprecise_dtypes=True)

            # ---- 3-5. rows ----
            for blk in range(HH // RB):
                xwb = io.tile([128, 6 * NW * C], BF16, tag="xwb")
                nc.sync.dma_start(
                    out=xwb[:],
                    in_=xwin_d[:, blk * RB * NW * C:(blk * RB + 6) * NW * C])
                pouts = []
                vts = []
                for ri in range(RB):
                    row = blk * RB + ri
                    mt = mpool.tile([128, N1], BF16, tag="m")
                    nc.vector.memset(mt[96:128, :], 0.0)
                    texp = mpool.tile([125, N1], BF16, tag="texp")
                    nc.sync.dma_start(
                        out=texp[:].rearrange("p (k c) -> p k c", k=K),
                        in_=t3[:, row, :].unsqueeze(0)
                        .broadcast_to([125, K, WJ]))
                    for d5 in range(ND):
                        nc.sync.dma_start(
                            out=mt[d5 * 25:(d5 + 1) * 25, :].rearrange(
                                "p (k c) -> p k c", k=K),
                            in_=a4[d5, :, row, :].unsqueeze(0)
                            .broadcast_to([25, K, WJ]))
                    sab = mpool.tile([125, N1], BF16, tag="sab")
                    nc.vector.scalar_tensor_tensor(
                        sab[:], texp[:], xlb[:], jkx[:],
                        op0=mybir.AluOpType.subtract, op1=ADD)
                    nc.scalar.activation(sab[:], sab[:],
                                         mybir.ActivationFunctionType.Abs,
                                         bias=zb125[:])
                    nc.vector.tensor_scalar(sab[:], sab[:], -1.0, 1.0,
                                            op0=MUL, op1=ADD)
                    nc.vector.scalar_tensor_tensor(
                        mt[0:125, :], sab[:], 0.0, mt[0:125, :],
                        op0=mybir.AluOpType.max, op1=MUL)
                    if debug and row == 0:
                        mdb = mpool.tile([125, N1], F32, tag="mdb")
                        nc.vector.tensor_copy(mdb[:], mt[0:125, :])
                        nc.sync.dma_start(out=dbg["M"][:], in_=mdb[:])
                    # stage-1: per-ky stationaries (window base row+ky-1)
                    vt = vpool.tile([C, N1], BF16, tag="v")
                    vts.append(vt)
                    mt3 = mt[:].rearrange("p (k wj) -> p k wj", k=K)
                    for w8 in range(NW):
                        pv = ps1.tile([C, K * CW], F32, tag="pv")
                        for ky in range(3):
                            ti = (ri + ky) * NW + w8
                            nc.tensor.matmul(
                                pv[:, ky * 3 * CW:(ky + 1) * 3 * CW],
                                xwb[:, ti * C:(ti + 1) * C],
                                mt3[:, 3 * ky:3 * ky + 3,
                                    w8 * CW:(w8 + 1) * CW],
                                start=(ky == 0), stop=(ky == 2))
                        dst = _ap(vt[:, w8 * CW:w8 * CW + 1], [[WJ, K], [1, CW]])
                        nc.vector.tensor_copy(
                            dst, pv[:].rearrange("c (k j) -> c k j", k=K))
                    if debug and row == 0:
                        vdb = vpool.tile([C, N1], F32, tag="vdb")
                        nc.vector.tensor_copy(vdb[:], vt[:])
                        nc.sync.dma_start(out=dbg["v"][:], in_=vdb[:])
                    pt = ps2.tile([O, WJ], F32, tag="pout")
                    pouts.append(pt)
                for k9 in range(K):
                    for ri in range(RB):
                        nc.tensor.matmul(
                            pouts[ri][:], w2r[:, k9 * O:(k9 + 1) * O],
                            vts[ri][:, k9 * WJ:(k9 + 1) * WJ],
                            start=(k9 == 0), stop=(k9 == K - 1))
                osb = io.tile([O, RB * 128], F32, tag="osb")
                for ri in range(RB):
                    nc.vector.tensor_scalar_add(
                        osb[:, ri * 128:(ri + 1) * 128],
                        pouts[ri][:, 0:128], bias[:])
                nc.sync.dma_start(
                    out=out_d[:, blk * RB * 128:(blk + 1) * RB * 128],
                    in_=osb[:])
    nc.compile()
    return nc


def host_prep(input_feat, inter, weight, bias, w_om, b_om):
    import ml_dtypes
    maps = []
    womr = np.ascontiguousarray(
        w_om.transpose(1, 2, 3, 0).reshape(2 * C, 9 * 27)).astype(np.float32)
    w2r = np.ascontiguousarray(
        weight.reshape(O, C, K).transpose(1, 2, 0).reshape(C, K * O)
    ).astype(ml_dtypes.bfloat16)
    kx3 = np.tile((3.0 + np.arange(9) % 3).astype(np.float32), 4)[:, None]
    xlb = (np.arange(125) % 25).astype(np.float32)[:, None]
    for b in range(B):
        xpad = np.zeros((C, H + 6, 144), np.float32)
        xpad[:, 3:3 + H, 4:4 + W] = input_feat[b]
        featb = np.concatenate([input_feat[b], inter[b]], axis=0)
        for half in range(2):
            h0 = half * HH
            fs = np.zeros((2 * C, 66, 130), np.float32)
            r_lo, r_hi = max(0, h0 - 1), min(H, h0 + 65)
            fs[:, r_lo - (h0 - 1):r_hi - (h0 - 1), 1:129] = featb[:, r_lo:r_hi]
            xs = xpad[:, h0:h0 + 70, :]                    # [C, 70, 144]
            rows = (np.arange(66)[:, None, None, None]
                    + np.arange(ND)[None, None, None, :])  # r + d
            cols = (17 * np.arange(NW)[None, :, None, None]
                    + np.arange(XW)[None, None, :, None])
            xw = xs[:, rows, cols]                         # [C,66,NW,XW,ND]
            xw = xw.transpose(4, 3, 1, 2, 0).reshape(125, 66 * NW * C)
            xwp = np.zeros((128, 66 * NW * C), np.float32)
            xwp[:125] = xw
            maps.append({
                "feat": fs.reshape(2 * C, 66 * 130),
                "xwin": xwp.astype(ml_dtypes.bfloat16),
                "womr": womr, "w2r": w2r,
                "bias": np.asarray(bias, np.float32).reshape(O, 1),
                "bom": np.asarray(b_om, np.float32).reshape(27, 1),
                "kx3": kx3, "xlb": xlb,
                "dbias": np.tile(-(np.arange(5, dtype=np.float32) - 2),
                                 (36, 1)),
                "zb125": np.zeros((125, 1), np.float32),
            })
    return maps


def kernel(input_feat, inter, weight, bias, w_om, b_om):
    if "nc" not in _cache:
        _cache["nc"] = build_bass(debug=False)
    nc = _cache["nc"]
    maps = host_prep(np.asarray(input_feat, np.float32),
                     np.asarray(inter, np.float32),
                     np.asarray(weight, np.float32),
                     np.asarray(bias, np.float32),
                     np.asarray(w_om, np.float32),
                     np.asarray(b_om, np.float32))
    res = run_bass_kernel_spmd(nc, maps, list(range(NCORES)))
    out = np.zeros((B, O, H, W), np.float32)
    for ci in range(NCORES):
        b, half = ci // 2, ci % 2
        out[b, :, half * HH:(half + 1) * HH] = \
            res.results[ci]["out"].reshape(O, HH, W)
    return out


# revision 12
# speedup vs baseline: 1.0012x; 1.0012x over previous
"""DCNv2 (modulated deformable conv) Trainium2 kernel.

8 cores = 4 batch samples x 2 image halves. Per core:
  1. Offset conv on PE (fp32): om[27, 8192].
  2. Hat planes (DVE/ACT): A[d][tap,pix] = relu(1-|dy-(d-2)|)*sigmoid(m),
     t[tap,pix136] = dx + 3 + kx + j. Round-trip DRAM for affine replication.
  3. Per row: M[125, 1224] = relu(1 - |x_l - t|) * A  (partition = x_l*5+d).
  4. Stage-1 matmuls: host-prebuilt x-window stationaries [125, 64] x M
     -> v[64, (k, 17w+j)] = exact bilinear samples * mask (|offset|<=2).
  5. Stage-2: out[o,.] += W_k.T @ v_k over 9 taps; bias on evac.
"""
import sys
sys.path.insert(0, "/opt/trn_rl_repo")
import numpy as np
import concourse.bass as bass
import concourse.tile as tile
from concourse import bacc, mybir
from concourse.bass_utils import run_bass_kernel_spmd

F32, BF16 = mybir.dt.float32, mybir.dt.bfloat16
MUL, ADD = mybir.AluOpType.mult, mybir.AluOpType.add

B, C, O, H, W = 4, 64, 128, 128, 128
K = 9
HH = 64
P = HH * W
CW, NW, XW, ND = 17, 8, 25, 5
WJ = NW * CW           # 136
N1 = K * WJ            # 1224
RB = 4                 # rows per stage-2 block
NCORES = 8

_cache = {}


def _ap(base, dims):
    """Manual AP: keep base partition dim, replace free dims."""
    return bass.AP(base.tensor, base.offset, [base.ap[0]] + dims)


def build_bass(debug=False):
    nc = bacc.Bacc("TRN2", target_bir_lowering=False, debug=False,
                   num_devices=NCORES)
    dp = lambda n, s, dt, out=False: nc.dram_tensor(
        n, s, dt, kind="ExternalOutput" if out else "ExternalInput").ap()

    feat_d = dp("feat", [2 * C, 66 * 130], F32)
    xwin_d = dp("xwin", [128, 66 * NW * C], BF16)
    womr_d = dp("womr", [2 * C, 9 * 27], F32)
    w2r_d = dp("w2r", [C, K * O], BF16)
    bias_d = dp("bias", [O, 1], F32)
    bom_d = dp("bom", [27, 1], F32)
    kx3_d = dp("kx3", [36, 1], F32)
    xlb_d = dp("xlb", [125, 1], F32)
    dbias_d = dp("dbias", [36, 5], F32)
    zb125_d = dp("zb125", [125, 1], F32)
    out_d = dp("out", [O, P], F32, out=True)
    dbg = {}
    if debug:
        dbg["om"] = dp("dbg_om", [27, P], F32, out=True)
        dbg["M"] = dp("dbg_M", [125, N1], F32, out=True)
        dbg["v"] = dp("dbg_v", [C, N1], F32, out=True)

    omdram = nc.dram_tensor("omdram", [27, P], F32).ap()
    adram = nc.dram_tensor("adram", [ND * K * HH * WJ], BF16).ap()
    tdram = nc.dram_tensor("tdram", [K * HH * WJ], BF16).ap()
    a4 = adram.rearrange("(d k r c) -> d k r c", d=ND, k=K, r=HH)
    t3 = tdram.rearrange("(k r c) -> k r c", k=K, r=HH)

    with tile.TileContext(nc) as tc:
        with (
            tc.tile_pool(name="big", bufs=1) as big,
            tc.tile_pool(name="io", bufs=3) as io,
            tc.tile_pool(name="mpool", bufs=3) as mpool,
            tc.tile_pool(name="vpool", bufs=6) as vpool,
            tc.tile_pool(name="ps1", bufs=2, space="PSUM") as ps1,
            tc.tile_pool(name="ps2", bufs=4, space="PSUM") as ps2,
            tc.tile_pool(name="small", bufs=1) as small,
        ):
            womr = small.tile([2 * C, 9 * 27], F32)
            nc.sync.dma_start(out=womr[:], in_=womr_d[:])
            w2r = small.tile([C, K * O], BF16)
            nc.sync.dma_start(out=w2r[:], in_=w2r_d[:])
            bias = small.tile([O, 1], F32)
            nc.sync.dma_start(out=bias[:], in_=bias_d[:])
            bom = small.tile([27, 1], F32)
            nc.sync.dma_start(out=bom[:], in_=bom_d[:])
            kx3 = small.tile([36, 1], F32)
            nc.sync.dma_start(out=kx3[:], in_=kx3_d[:])
            xlb = small.tile([125, 1], F32)
            nc.sync.dma_start(out=xlb[:], in_=xlb_d[:])
            dbias = small.tile([36, 5], F32)
            nc.sync.dma_start(out=dbias[:], in_=dbias_d[:])
            zb125 = small.tile([125, 1], F32)
            nc.sync.dma_start(out=zb125[:], in_=zb125_d[:])

            feat = big.tile([2 * C, 66 * 130], F32)
            nc.sync.dma_start(out=feat[:], in_=feat_d[:])

            # ---- 1. offset conv ----
            for ch in range(16):
                r0 = ch * 4
                pom = ps1.tile([27, 512], F32, tag="pom")
                for t9 in range(9):
                    dy, dx = t9 // 3, t9 % 3
                    base = feat[:, (r0 + dy) * 130 + dx:(r0 + dy) * 130 + dx + 1]
                    rhs = _ap(base, [[130, 4], [1, 128]])
                    nc.tensor.matmul(pom[:], womr[:, t9 * 27:(t9 + 1) * 27],
                                     rhs, start=(t9 == 0), stop=(t9 == 8))
                omc = io.tile([27, 512], F32, tag="omc")
                nc.vector.tensor_scalar_add(omc[:], pom[:], bom[:])
                nc.sync.dma_start(out=omdram[:, ch * 512:(ch + 1) * 512],
                                  in_=omc[:])
            if debug:
                nc.sync.dma_start(out=dbg["om"][:], in_=omdram[:])

            # ---- 2. fat planes [36, 2056] ----
            dyf = big.tile([36, 2056], F32)
            dxf = big.tile([36, 2056], F32)
            msf = big.tile([36, 2056], F32)
            for f in (dyf, dxf, msf):
                nc.vector.memset(f[:], 0.0)
            for q in range(4):
                pix = slice(q * 2048, (q + 1) * 2048)
                nc.sync.dma_start(out=dyf[q * 9:(q + 1) * 9, 0:2048],
                                  in_=omdram[0:18:2, pix])
                nc.sync.dma_start(out=dxf[q * 9:(q + 1) * 9, 0:2048],
                                  in_=omdram[1:18:2, pix])
                nc.sync.dma_start(out=msf[q * 9:(q + 1) * 9, 0:2048],
                                  in_=omdram[18:27, pix])
            nc.scalar.activation(msf[:, 0:2048], msf[:, 0:2048],
                                 mybir.ActivationFunctionType.Sigmoid,
                                 bias=dbias[:, 2:3])
            nc.vector.memset(msf[:, 2048:2056], 0.0)

            # t136 = dx (136-layout) -> tdram (bf16); integer part added later
            dxv = _ap(dxf[:], [[128, 16], [17, 8], [1, 17]])
            t136b = big.tile([36, 2176], BF16)
            nc.vector.tensor_copy(t136b[:].rearrange(
                "p (a b c) -> p a b c", a=16, b=8, c=17), dxv)
            for q in range(4):
                nc.sync.dma_start(
                    out=t3[:, q * 16:(q + 1) * 16, :],
                    in_=t136b[q * 9:(q + 1) * 9, :].rearrange(
                        "p (r c) -> p r c", r=16))

            # A planes -> adram (bf16)
            msv = _ap(msf[:], [[128, 16], [17, 8], [1, 17]])
            for d5 in range(ND):
                ab = big.tile([36, 2176], F32, tag="aplane")
                dyv = _ap(dyf[:], [[128, 16], [17, 8], [1, 17]])
                ab4 = ab[:].rearrange("p (a b c) -> p a b c", a=16, b=8, c=17)
                nc.scalar.activation(ab4, dyv,
                                     mybir.ActivationFunctionType.Abs,
                                     bias=dbias[:, d5:d5 + 1], scale=1.0)
                nc.vector.tensor_scalar(ab[:], ab[:], -1.0, 1.0,
                                        op0=MUL, op1=ADD)
                nc.vector.tensor_scalar_max(ab[:], ab[:], 0.0)
                abb = big.tile([36, 2176], BF16, tag="aplaneb")
                abb4 = abb[:].rearrange("p (a b c) -> p a b c", a=16, b=8, c=17)
                nc.vector.tensor_tensor(abb4, ab4, msv, op=MUL)
                for q in range(4):
                    nc.sync.dma_start(
                        out=a4[d5, :, q * 16:(q + 1) * 16, :],
                        in_=abb[q * 9:(q + 1) * 9, :].rearrange(
                            "p (r c) -> p r c", r=16))

            jkx = big.tile([125, N1], BF16)
            nc.gpsimd.iota(jkx[:], pattern=[[0, 3], [1, 3], [0, 8], [1, 17]],
                           base=3, channel_multiplier=0,
                           allow_small_or_imprecise_dtypes=True)

            # ---- 3-5. rows ----
            for blk in range(HH // RB):
                xwb = io.tile([128, 6 * NW * C], BF16, tag="xwb")
                nc.sync.dma_start(
                    out=xwb[:],
                    in_=xwin_d[:, blk * RB * NW * C:(blk * RB + 6) * NW * C])
                pouts = []
                vts = []
                for ri in range(RB):
                    row = blk * RB + ri
                    mt = mpool.tile([128, N1], BF16, tag="m")
                    nc.vector.memset(mt[96:128, :], 0.0)
                    texp = mpool.tile([125, N1], BF16, tag="texp")
                    nc.sync.dma_start(
                        out=texp[:].rearrange("p (k c) -> p k c", k=K),
                        in_=t3[:, row, :].unsqueeze(0)
                        .broadcast_to([125, K, WJ]))
                    for d5 in range(ND):
                        nc.sync.dma_start(
                            out=mt[d5 * 25:(d5 + 1) * 25, :].rearrange(
                                "p (k c) -> p k c", k=K),
                            in_=a4[d5, :, row, :].unsqueeze(0)
                            .broadcast_to([25, K, WJ]))
                    sab = mpool.tile([125, N1], BF16, tag="sab")
                    nc.vector.scalar_tensor_tensor(
                        sab[:], texp[:], xlb[:], jkx[:],
                        op0=mybir.AluOpType.subtract, op1=ADD)
                    nc.scalar.activation(sab[:], sab[:],
                                         mybir.ActivationFunctionType.Abs,
                                         bias=zb125[:])
                    nc.vector.tensor_scalar(sab[:], sab[:], -1.0, 1.0,
                                            op0=MUL, op1=ADD)
                    nc.vector.scalar_tensor_tensor(
                        mt[0:125, :], sab[:], 0.0, mt[0:125, :],
                        op0=mybir.AluOpType.max, op1=MUL)
                    if debug and row == 0:
                        mdb = mpool.tile([125, N1], F32, tag="mdb")
                        nc.vector.tensor_copy(mdb[:], mt[0:125, :])
                        nc.sync.dma_start(out=dbg["M"][:], in_=mdb[:])
                    # stage-1: per-ky stationaries (window base row+ky-1)
                    vt = vpool.tile([C, N1], BF16, tag="v")
                    vts.append(vt)
                    mt3 = mt[:].rearrange("p (k wj) -> p k wj", k=K)
                    for w8 in range(NW):
                        pv = ps1.tile([C, K * CW], F32, tag="pv")
                        for ky in range(3):
                            ti = (ri + ky) * NW + w8
                            nc.tensor.matmul(
                                pv[:, ky * 3 * CW:(ky + 1) * 3 * CW],
                                xwb[:, ti * C:(ti + 1) * C],
                                mt3[:, 3 * ky:3 * ky + 3,
                                    w8 * CW:(w8 + 1) * CW],
                                start=(ky == 0), stop=(ky == 2))
                        dst = _ap(vt[:, w8 * CW:w8 * CW + 1], [[WJ, K], [1, CW]])
                        nc.vector.tensor_copy(
                            dst, pv[:].rearrange("c (k j) -> c k j", k=K))
                    if debug and row == 0:
                        vdb = vpool.tile([C, N1], F32, tag="vdb")
                        nc.vector.tensor_copy(vdb[:], vt[:])
                        nc.sync.dma_start(out=dbg["v"][:], in_=vdb[:])
                    pt = ps2.tile([O, WJ], F32, tag="pout")
                    pouts.append(pt)
                for k9 in range(K):
                    for ri in range(RB):
                        nc.tensor.matmul(
                            pouts[ri][:], w2r[:, k9 * O:(k9 + 1) * O],
                            vts[ri][:, k9 * WJ:(k9 + 1) * WJ],
                            start=(k9 == 0), stop=(k9 == K - 1))
                osb = io.tile([O, RB * 128], F32, tag="osb")
                for ri in range(RB):
                    nc.vector.tensor_scalar_add(
                        osb[:, ri * 128:(ri + 1) * 128],
                        pouts[ri][:, 0:128], bias[:])
                nc.sync.dma_start(
                    out=out_d[:, blk * RB * 128:(blk + 1) * RB * 128],
                    in_=osb[:])
    nc.compile()
    return nc


def host_prep(input_feat, inter, weight, bias, w_om, b_om):
    import ml_dtypes
    maps = []
    womr = np.ascontiguousarray(
        w_om.transpose(1, 2, 3, 0).reshape(2 * C, 9 * 27)).astype(np.float32)
    w2r = np.ascontiguousarray(
        weight.reshape(O, C, K).transpose(1, 2, 0).reshape(C, K * O)
    ).astype(ml_dtypes.bfloat16)
    kx3 = np.tile((3.0 + np.arange(9) % 3).astype(np.float32), 4)[:, None]
    xlb = (np.arange(125) % 25).astype(np.float32)[:, None]
    for b in range(B):
        xpad = np.zeros((C, H + 6, 144), np.float32)
        xpad[:, 3:3 + H, 4:4 + W] = input_feat[b]
        featb = np.concatenate([input_feat[b], inter[b]], axis=0)
        for half in range(2):
            h0 = half * HH
            fs = np.zeros((2 * C, 66, 130), np.float32)
            r_lo, r_hi = max(0, h0 - 1), min(H, h0 + 65)
            fs[:, r_lo - (h0 - 1):r_hi - (h0 - 1), 1:129] = featb[:, r_lo:r_hi]
            xs = xpad[:, h0:h0 + 70, :]                    # [C, 70, 144]
            rows = (np.arange(66)[:, None, None, None]
                    + np.arange(ND)[None, None, None, :])  # r + d
            cols = (17 * np.arange(NW)[None, :, None, None]
                    + np.arange(XW)[None, None, :, None])
            xw = xs[:, rows, cols]                         # [C,66,NW,XW,ND]
            xw = xw.transpose(4, 3, 1, 2, 0).reshape(125, 66 * NW * C)
            xwp = np.zeros((128, 66 * NW * C), np.float32)
            xwp[:125] = xw
            maps.append({
                "feat": fs.reshape(2 * C, 66 * 130),
                "xwin": xwp.astype(ml_dtypes.bfloat16),
                "womr": womr, "w2r": w2r,
                "bias": np.asarray(bias, np.float32).reshape(O, 1),
                "bom": np.asarray(b_om, np.float32).reshape(27, 1),
                "kx3": kx3, "xlb": xlb,
                "dbias": np.tile(-(np.arange(5, dtype=np.float32) - 2),
                                 (36, 1)),
                "zb125": np.zeros((125, 1), np.float32),
            })
    return maps


def kernel(input_feat, inter, weight, bias, w_om, b_om):
    if "nc" not in _cache:
        _cache["nc"] = build_bass(debug=False)
    nc = _cache["nc"]
    maps = host_prep(np.asarray(input_feat, np.float32),
                     np.asarray(inter, np.float32),
                     np.asarray(weight, np.float32),
                     np.asarray(bias, np.float32),
                     np.asarray(w_om, np.float32),
                     np.asarray(b_om, np.float32))
    res = run_bass_kernel_spmd(nc, maps, list(range(NCORES)))
    out = np.zeros((B, O, H, W), np.float32)
    for ci in range(NCORES):
        b, half = ci // 2, ci % 2
        out[b, :, half * HH:(half + 1) * HH] = \
            res.results[ci]["out"].reshape(O, HH, W)
    return out


# revision 13
# speedup vs baseline: 1.0645x; 1.0632x over previous
"""DCNv2 (modulated deformable conv) Trainium2 kernel.

8 cores = 4 batch samples x 2 image halves. Per core:
  1. Offset conv on PE (fp32): om[27, 8192].
  2. Hat planes (DVE/ACT): A[d][tap,pix] = relu(1-|dy-(d-2)|)*sigmoid(m),
     t[tap,pix136] = dx + 3 + kx + j. Round-trip DRAM for affine replication.
  3. Per row: M[125, 1224] = relu(1 - |x_l - t|) * A  (partition = x_l*5+d).
  4. Stage-1 matmuls: host-prebuilt x-window stationaries [125, 64] x M
     -> v[64, (k, 17w+j)] = exact bilinear samples * mask (|offset|<=2).
  5. Stage-2: out[o,.] += W_k.T @ v_k over 9 taps; bias on evac.
"""
import sys
sys.path.insert(0, "/opt/trn_rl_repo")
import numpy as np
import concourse.bass as bass
import concourse.tile as tile
from concourse import bacc, mybir
from concourse.bass_utils import run_bass_kernel_spmd

F32, BF16 = mybir.dt.float32, mybir.dt.bfloat16
MUL, ADD = mybir.AluOpType.mult, mybir.AluOpType.add

B, C, O, H, W = 4, 64, 128, 128, 128
K = 9
HH = 64
P = HH * W
CW, NW, XW, ND = 17, 8, 25, 5
WJ = NW * CW           # 136
N1 = K * WJ            # 1224
RB = 4                 # rows per stage-2 block
NCORES = 8

_cache = {}


def _ap(base, dims):
    """Manual AP: keep base partition dim, replace free dims."""
    return bass.AP(base.tensor, base.offset, [base.ap[0]] + dims)


def build_bass(debug=False):
    nc = bacc.Bacc("TRN2", target_bir_lowering=False, debug=False,
                   num_devices=NCORES)
    dp = lambda n, s, dt, out=False: nc.dram_tensor(
        n, s, dt, kind="ExternalOutput" if out else "ExternalInput").ap()

    feat_d = dp("feat", [2 * C, 66 * 130], F32)
    xwin_d = dp("xwin", [128, 66 * NW * C], BF16)
    womr_d = dp("womr", [2 * C, 9 * 27], F32)
    w2r_d = dp("w2r", [C, K * O], BF16)
    bias_d = dp("bias", [O, 1], F32)
    bom_d = dp("bom", [27, 1], F32)
    kx3_d = dp("kx3", [36, 1], F32)
    xlb_d = dp("xlb", [125, 1], F32)
    dbias_d = dp("dbias", [36, 5], F32)
    zb125_d = dp("zb125", [125, 1], F32)
    zb125p1_d = dp("zb125p1", [125, 1], F32)
    out_d = dp("out", [O, P], F32, out=True)
    dbg = {}
    if debug:
        dbg["om"] = dp("dbg_om", [27, P], F32, out=True)
        dbg["M"] = dp("dbg_M", [125, N1], F32, out=True)
        dbg["v"] = dp("dbg_v", [C, N1], F32, out=True)

    omdram = nc.dram_tensor("omdram", [27, P], F32).ap()
    adram = nc.dram_tensor("adram", [ND * K * HH * WJ], BF16).ap()
    tdram = nc.dram_tensor("tdram", [K * HH * WJ], BF16).ap()
    a4 = adram.rearrange("(d k r c) -> d k r c", d=ND, k=K, r=HH)
    t3 = tdram.rearrange("(k r c) -> k r c", k=K, r=HH)

    with tile.TileContext(nc) as tc:
        with (
            tc.tile_pool(name="big", bufs=1) as big,
            tc.tile_pool(name="io", bufs=3) as io,
            tc.tile_pool(name="mpool", bufs=3) as mpool,
            tc.tile_pool(name="vpool", bufs=6) as vpool,
            tc.tile_pool(name="ps1", bufs=2, space="PSUM") as ps1,
            tc.tile_pool(name="ps2", bufs=4, space="PSUM") as ps2,
            tc.tile_pool(name="small", bufs=1) as small,
        ):
            womr = small.tile([2 * C, 9 * 27], F32)
            nc.sync.dma_start(out=womr[:], in_=womr_d[:])
            w2r = small.tile([C, K * O], BF16)
            nc.sync.dma_start(out=w2r[:], in_=w2r_d[:])
            bias = small.tile([O, 1], F32)
            nc.sync.dma_start(out=bias[:], in_=bias_d[:])
            bom = small.tile([27, 1], F32)
            nc.sync.dma_start(out=bom[:], in_=bom_d[:])
            kx3 = small.tile([36, 1], F32)
            nc.sync.dma_start(out=kx3[:], in_=kx3_d[:])
            xlb = small.tile([125, 1], F32)
            nc.sync.dma_start(out=xlb[:], in_=xlb_d[:])
            dbias = small.tile([36, 5], F32)
            nc.sync.dma_start(out=dbias[:], in_=dbias_d[:])
            zb125 = small.tile([125, 1], F32)
            nc.sync.dma_start(out=zb125[:], in_=zb125_d[:])
            zb125p1 = small.tile([125, 1], F32)
            nc.sync.dma_start(out=zb125p1[:], in_=zb125p1_d[:])

            feat = big.tile([2 * C, 66 * 130], F32)
            nc.sync.dma_start(out=feat[:], in_=feat_d[:])

            # ---- 1. offset conv ----
            for ch in range(16):
                r0 = ch * 4
                pom = ps1.tile([27, 512], F32, tag="pom")
                for t9 in range(9):
                    dy, dx = t9 // 3, t9 % 3
                    base = feat[:, (r0 + dy) * 130 + dx:(r0 + dy) * 130 + dx + 1]
                    rhs = _ap(base, [[130, 4], [1, 128]])
                    nc.tensor.matmul(pom[:], womr[:, t9 * 27:(t9 + 1) * 27],
                                     rhs, start=(t9 == 0), stop=(t9 == 8))
                omc = io.tile([27, 512], F32, tag="omc")
                nc.vector.tensor_scalar_add(omc[:], pom[:], bom[:])
                nc.sync.dma_start(out=omdram[:, ch * 512:(ch + 1) * 512],
                                  in_=omc[:])
            if debug:
                nc.sync.dma_start(out=dbg["om"][:], in_=omdram[:])

            # ---- 2. fat planes [36, 2056] ----
            dyf = big.tile([36, 2056], F32)
            dxf = big.tile([36, 2056], F32)
            msf = big.tile([36, 2056], F32)
            for f in (dyf, dxf, msf):
                nc.vector.memset(f[:], 0.0)
            for q in range(4):
                pix = slice(q * 2048, (q + 1) * 2048)
                nc.sync.dma_start(out=dyf[q * 9:(q + 1) * 9, 0:2048],
                                  in_=omdram[0:18:2, pix])
                nc.sync.dma_start(out=dxf[q * 9:(q + 1) * 9, 0:2048],
                                  in_=omdram[1:18:2, pix])
                nc.sync.dma_start(out=msf[q * 9:(q + 1) * 9, 0:2048],
                                  in_=omdram[18:27, pix])
            nc.scalar.activation(msf[:, 0:2048], msf[:, 0:2048],
                                 mybir.ActivationFunctionType.Sigmoid,
                                 bias=dbias[:, 2:3])
            nc.vector.memset(msf[:, 2048:2056], 0.0)

            # t136 = dx (136-layout) -> tdram (bf16); integer part added later
            dxv = _ap(dxf[:], [[128, 16], [17, 8], [1, 17]])
            t136b = big.tile([36, 2176], BF16)
            nc.vector.tensor_copy(t136b[:].rearrange(
                "p (a b c) -> p a b c", a=16, b=8, c=17), dxv)
            for q in range(4):
                nc.sync.dma_start(
                    out=t3[:, q * 16:(q + 1) * 16, :],
                    in_=t136b[q * 9:(q + 1) * 9, :].rearrange(
                        "p (r c) -> p r c", r=16))

            # A planes -> adram (bf16)
            msv = _ap(msf[:], [[128, 16], [17, 8], [1, 17]])
            for d5 in range(ND):
                ab = big.tile([36, 2176], F32, tag="aplane")
                dyv = _ap(dyf[:], [[128, 16], [17, 8], [1, 17]])
                ab4 = ab[:].rearrange("p (a b c) -> p a b c", a=16, b=8, c=17)
                nc.scalar.activation(ab4, dyv,
                                     mybir.ActivationFunctionType.Abs,
                                     bias=dbias[:, d5:d5 + 1], scale=1.0)
                nc.vector.tensor_scalar(ab[:], ab[:], -1.0, 1.0,
                                        op0=MUL, op1=ADD)
                nc.vector.tensor_scalar_max(ab[:], ab[:], 0.0)
                abb = big.tile([36, 2176], BF16, tag="aplaneb")
                abb4 = abb[:].rearrange("p (a b c) -> p a b c", a=16, b=8, c=17)
                nc.vector.tensor_tensor(abb4, ab4, msv, op=MUL)
                for q in range(4):
                    nc.sync.dma_start(
                        out=a4[d5, :, q * 16:(q + 1) * 16, :],
                        in_=abb[q * 9:(q + 1) * 9, :].rearrange(
                            "p (r c) -> p r c", r=16))

            jkx = big.tile([125, N1], BF16)
            nc.gpsimd.iota(jkx[:], pattern=[[0, 3], [1, 3], [0, 8], [1, 17]],
                           base=3, channel_multiplier=0,
                           allow_small_or_imprecise_dtypes=True)

            # ---- 3-5. rows ----
            for blk in range(HH // RB):
                xwb = io.tile([128, 6 * NW * C], BF16, tag="xwb")
                nc.sync.dma_start(
                    out=xwb[:],
                    in_=xwin_d[:, blk * RB * NW * C:(blk * RB + 6) * NW * C])
                pouts = []
                vts = []
                for ri in range(RB):
                    row = blk * RB + ri
                    mt = mpool.tile([128, N1], BF16, tag="m")
                    nc.vector.memset(mt[96:128, :], 0.0)
                    texp = mpool.tile([125, N1], BF16, tag="texp")
                    nc.sync.dma_start(
                        out=texp[:].rearrange("p (k c) -> p k c", k=K),
                        in_=t3[:, row, :].unsqueeze(0)
                        .broadcast_to([125, K, WJ]))
                    for d5 in range(ND):
                        nc.sync.dma_start(
                            out=mt[d5 * 25:(d5 + 1) * 25, :].rearrange(
                                "p (k c) -> p k c", k=K),
                            in_=a4[d5, :, row, :].unsqueeze(0)
                            .broadcast_to([25, K, WJ]))
                    sab = mpool.tile([125, N1], BF16, tag="sab")
                    nc.vector.scalar_tensor_tensor(
                        sab[:], texp[:], xlb[:], jkx[:],
                        op0=mybir.AluOpType.subtract, op1=ADD)
                    nc.scalar.activation(sab[:], sab[:],
                                         mybir.ActivationFunctionType.Abs,
                                         bias=zb125[:])
                    nc.scalar.activation(sab[:], sab[:],
                                         mybir.ActivationFunctionType.Relu,
                                         bias=zb125p1[:], scale=-1.0)
                    nc.vector.tensor_tensor(mt[0:125, :], mt[0:125, :],
                                            sab[:], op=MUL)
                    if debug and row == 0:
                        mdb = mpool.tile([125, N1], F32, tag="mdb")
                        nc.vector.tensor_copy(mdb[:], mt[0:125, :])
                        nc.sync.dma_start(out=dbg["M"][:], in_=mdb[:])
                    # stage-1: per-ky stationaries (window base row+ky-1)
                    vt = vpool.tile([C, N1], BF16, tag="v")
                    vts.append(vt)
                    mt3 = mt[:].rearrange("p (k wj) -> p k wj", k=K)
                    for w8 in range(NW):
                        pv = ps1.tile([C, K * CW], F32, tag="pv")
                        for ky in range(3):
                            ti = (ri + ky) * NW + w8
                            nc.tensor.matmul(
                                pv[:, ky * 3 * CW:(ky + 1) * 3 * CW],
                                xwb[:, ti * C:(ti + 1) * C],
                                mt3[:, 3 * ky:3 * ky + 3,
                                    w8 * CW:(w8 + 1) * CW],
                                start=(ky == 0), stop=(ky == 2))
                        dst = _ap(vt[:, w8 * CW:w8 * CW + 1], [[WJ, K], [1, CW]])
                        nc.vector.tensor_copy(
                            dst, pv[:].rearrange("c (k j) -> c k j", k=K))
                    if debug and row == 0:
                        vdb = vpool.tile([C, N1], F32, tag="vdb")
                        nc.vector.tensor_copy(vdb[:], vt[:])
                        nc.sync.dma_start(out=dbg["v"][:], in_=vdb[:])
                    pt = ps2.tile([O, WJ], F32, tag="pout")
                    pouts.append(pt)
                for k9 in range(K):
                    for ri in range(RB):
                        nc.tensor.matmul(
                            pouts[ri][:], w2r[:, k9 * O:(k9 + 1) * O],
                            vts[ri][:, k9 * WJ:(k9 + 1) * WJ],
                            start=(k9 == 0), stop=(k9 == K - 1))
                osb = io.tile([O, RB * 128], F32, tag="osb")
                for ri in range(RB):
                    nc.vector.tensor_scalar_add(
                        osb[:, ri * 128:(ri + 1) * 128],
                        pouts[ri][:, 0:128], bias[:])
                nc.sync.dma_start(
                    out=out_d[:, blk * RB * 128:(blk + 1) * RB * 128],
                    in_=osb[:])
    nc.compile()
    return nc


def host_prep(input_feat, inter, weight, bias, w_om, b_om):
    import ml_dtypes
    maps = []
    womr = np.ascontiguousarray(
        w_om.transpose(1, 2, 3, 0).reshape(2 * C, 9 * 27)).astype(np.float32)
    w2r = np.ascontiguousarray(
        weight.reshape(O, C, K).transpose(1, 2, 0).reshape(C, K * O)
    ).astype(ml_dtypes.bfloat16)
    kx3 = np.tile((3.0 + np.arange(9) % 3).astype(np.float32), 4)[:, None]
    xlb = (np.arange(125) % 25).astype(np.float32)[:, None]
    for b in range(B):
        xpad = np.zeros((C, H + 6, 144), np.float32)
        xpad[:, 3:3 + H, 4:4 + W] = input_feat[b]
        featb = np.concatenate([input_feat[b], inter[b]], axis=0)
        for half in range(2):
            h0 = half * HH
            fs = np.zeros((2 * C, 66, 130), np.float32)
            r_lo, r_hi = max(0, h0 - 1), min(H, h0 + 65)
            fs[:, r_lo - (h0 - 1):r_hi - (h0 - 1), 1:129] = featb[:, r_lo:r_hi]
            xs = xpad[:, h0:h0 + 70, :]                    # [C, 70, 144]
            rows = (np.arange(66)[:, None, None, None]
                    + np.arange(ND)[None, None, None, :])  # r + d
            cols = (17 * np.arange(NW)[None, :, None, None]
                    + np.arange(XW)[None, None, :, None])
            xw = xs[:, rows, cols]                         # [C,66,NW,XW,ND]
            xw = xw.transpose(4, 3, 1, 2, 0).reshape(125, 66 * NW * C)
            xwp = np.zeros((128, 66 * NW * C), np.float32)
            xwp[:125] = xw
            maps.append({
                "feat": fs.reshape(2 * C, 66 * 130),
                "xwin": xwp.astype(ml_dtypes.bfloat16),
                "womr": womr, "w2r": w2r,
                "bias": np.asarray(bias, np.float32).reshape(O, 1),
                "bom": np.asarray(b_om, np.float32).reshape(27, 1),
                "kx3": kx3, "xlb": xlb,
                "dbias": np.tile(-(np.arange(5, dtype=np.float32) - 2),
                                 (36, 1)),
                "zb125": np.zeros((125, 1), np.float32),
                "zb125p1": np.ones((125, 1), np.float32),
            })
    return maps


def kernel(input_feat, inter, weight, bias, w_om, b_om):
    if "nc" not in _cache:
        _cache["nc"] = build_bass(debug=False)
    nc = _cache["nc"]
    maps = host_prep(np.asarray(input_feat, np.float32),
                     np.asarray(inter, np.float32),
                     np.asarray(weight, np.float32),
                     np.asarray(bias, np.float32),
                     np.asarray(w_om, np.float32),
                     np.asarray(b_om, np.float32))
    res = run_bass_kernel_spmd(nc, maps, list(range(NCORES)))
    out = np.zeros((B, O, H, W), np.float32)
    for ci in range(NCORES):
        b, half = ci // 2, ci % 2
        out[b, :, half * HH:(half + 1) * HH] = \
            res.results[ci]["out"].reshape(O, HH, W)
    return out
